# revision 57
# baseline (speedup 1.0000x reference)
"""DDiT block (adaLN attention + MLP) on 8 Trainium2 NeuronCores.

Sharding: cores 0-3 -> batch 0, cores 4-7 -> batch 1. Within a 4-core
batch group: attention is sharded by heads (4 heads/core, full sequence);
after the attention out-projection a grouped ReduceScatter sums the
per-head partial outputs and hands each core a 512-token slice, on which
it runs the (token-sharded) MLP.

Host prep folds the adaLN modulation into weights/biases:
  - ada = c @ ada_w.T + ada_b is computed on host (12 MFLOP)
  - LN scale A = norm_w * (1 + sc); the shift's contribution to each
    linear layer is folded into that layer's bias (B @ W.T)
  - gates g_msa / g_mlp are folded into w_out / mlp_w2 rows
All weights are shipped pre-transposed ([d_in, d_out]) in bf16, so the
device only ever runs natural lhsT.T @ rhs matmuls.

Device pipeline per core: token-major LN1 (bn_stats) -> PE-transpose ->
q,k feature-major + v token-major projections -> per head: scoresT =
K@Q.T (2-head packed via tile_position), exp on ScalarE (no max
subtraction; scores are bounded), attn@V with a ones-augmented V giving
the softmax denominator for free, delayed division -> out-projection ->
ReduceScatter -> residual + LN2 -> MLP (gelu bias-folded) -> residual.
"""

import numpy as np

import concourse.bass as bass
import concourse.mybir as mybir
import concourse.tile as tile
from concourse import bacc
from concourse.bass_utils import run_bass_kernel_spmd
from concourse.masks import make_identity

B, S, D, H, HD = 2, 2048, 1024, 16, 64
DFF = 4 * D
TOK = S // 4          # tokens per core for the MLP phase
EPS = 1e-5
GROUPS = [[0, 1, 2, 3], [4, 5, 6, 7]]
F32 = mybir.dt.float32
BF16 = mybir.dt.bfloat16
F8 = mybir.dt.float8e4
DROW = mybir.MatmulPerfMode.DoubleRow
AF = mybir.ActivationFunctionType
ALU = mybir.AluOpType

_CACHE = {}


# ---------------------------------------------------------------- host prep

def _f(v):
    return np.ascontiguousarray(np.asarray(v, dtype=np.float32))


def _bf(a):
    import ml_dtypes
    return np.ascontiguousarray(a.astype(ml_dtypes.bfloat16))


def host_prep(inp):
    x, c = _f(inp["x"]), _f(inp["c"])
    norm1_w, norm2_w = _f(inp["norm1_w"]), _f(inp["norm2_w"])
    w_qkv, w_out = _f(inp["w_qkv"]), _f(inp["w_out"])
    mlp_w1, mlp_b1 = _f(inp["mlp_w1"]), _f(inp["mlp_b1"])
    mlp_w2, mlp_b2 = _f(inp["mlp_w2"]), _f(inp["mlp_b2"])
    ada_w, ada_b = _f(inp["ada_w"]), _f(inp["ada_b"])

    ada = c @ ada_w.T + ada_b                      # [B, 6D]
    sh_msa, sc_msa, g_msa, sh_mlp, sc_mlp, g_mlp = np.split(ada, 6, axis=1)
    A1 = norm1_w[None] * (1.0 + sc_msa)            # [B, D]
    A2 = norm2_w[None] * (1.0 + sc_mlp)
    bias_qkv = sh_msa @ w_qkv.T                    # [B, 3D]
    bias1 = mlp_b1[None] + sh_mlp @ mlp_w1.T       # [B, DFF]
    bias2 = g_mlp * mlp_b2[None]                   # [B, D]

    # LN1 on host (pure input prep): h1 = (x - mu)*rstd * A1, transposed
    mu1 = x.mean(-1, keepdims=True)
    rstd1 = 1.0 / np.sqrt(x.var(-1, keepdims=True) + EPS)
    h1 = (x - mu1) * rstd1 * A1[:, None, :]        # [B, S, D]
    h1T = [_bf(h1[b].T.copy()) for b in range(B)]  # [D, S] bf16 per batch

    wq, wk, wv = w_qkv[0:D], w_qkv[D:2 * D], w_qkv[2 * D:3 * D]
    w1T = _bf(mlp_w1.T.copy())                     # [D, DFF]
    # rt-major blocks [32, D, 128] so each rt's 8 lhsT chunks DMA contiguously
    w1blk = np.ascontiguousarray(
        w1T.reshape(D, 32, 128).transpose(1, 0, 2))  # [32, D, 128] bf16

    in_maps = []
    for cid in range(8):
        b, r = cid // 4, cid % 4
        hsl = slice(256 * r, 256 * r + 256)
        woutg = g_msa[b][:, None] * w_out          # [D, D]
        w2g = g_mlp[b][:, None] * mlp_w2           # [D, DFF]
        in_maps.append({
            "xnT": h1T[b],                                         # [D, S]
            "x_res": _bf(np.concatenate(
                [x[b][512 * t2 + 128 * r:512 * t2 + 128 * r + 128]
                 for t2 in range(4)])),
            "a2": _bf(A2[b]),
            "wqkT": _bf(np.vstack([wq[hsl], wk[hsl]]).T.copy()),   # [D, 512]
            "bias_qk": np.ascontiguousarray(np.concatenate(
                [bias_qkv[b, hsl],
                 bias_qkv[b, D + 256 * r:D + 256 * r + 256]])),    # [512]
            "wvT": _bf(wv[hsl].T.copy()),                          # [D, 256]
            "bias_v": np.ascontiguousarray(
                bias_qkv[b, 2 * D + 256 * r:2 * D + 256 * r + 256]),
            "woutT": _bf(woutg[:, hsl].T.copy()),                  # [256, D]
            "w1blk": w1blk,                                        # [32, D, 128]
            "bias1": np.ascontiguousarray(bias1[b]),
            "w2gT": _bf(w2g.T.copy()),                             # [DFF, D]
            "bias2": _bf(bias2[b]),
        })
    return in_maps


# ------------------------------------------------------------- device build

def _bc(ap, p=128):
    """Broadcast a DRAM row AP across p partitions (stride-0 partition dim)."""
    return bass.AP(tensor=ap.tensor, offset=ap.offset,
                   ap=[[0, p]] + [list(d) for d in ap.ap])


def build_program(reps=1):
    nc = bacc.Bacc("TRN2", target_bir_lowering=False, debug=False, num_devices=8)

    x_d = nc.dram_tensor("xnT", [D, S], BF16, kind="ExternalInput")
    xr_d = nc.dram_tensor("x_res", [TOK, D], BF16, kind="ExternalInput")
    a2_d = nc.dram_tensor("a2", [D], BF16, kind="ExternalInput")
    wqk_d = nc.dram_tensor("wqkT", [D, 512], BF16, kind="ExternalInput")
    bqk_d = nc.dram_tensor("bias_qk", [512], F32, kind="ExternalInput")
    wv_d = nc.dram_tensor("wvT", [D, 256], BF16, kind="ExternalInput")
    bv_d = nc.dram_tensor("bias_v", [256], F32, kind="ExternalInput")
    wo_d = nc.dram_tensor("woutT", [256, D], BF16, kind="ExternalInput")
    w1_d = nc.dram_tensor("w1blk", [32, D, 128], BF16, kind="ExternalInput")
    b1_d = nc.dram_tensor("bias1", [DFF], F32, kind="ExternalInput")
    w2_d = nc.dram_tensor("w2gT", [DFF, D], BF16, kind="ExternalInput")
    b2_d = nc.dram_tensor("bias2", [D], BF16, kind="ExternalInput")
    out_d = nc.dram_tensor("out", [TOK, D], F32, kind="ExternalOutput")

    with tile.TileContext(nc, num_cores=8) as tc:
        for _ in range(reps):
            _body(nc, tc, x_d, xr_d, a2_d, wqk_d, bqk_d, wv_d, bv_d,
                  wo_d, w1_d, b1_d, w2_d, b2_d, out_d)
    nc.compile()
    return nc


def _body(nc, tc, x_d, xr_d, a2_d, wqk_d, bqk_d, wv_d, bv_d,
          wo_d, w1_d, b1_d, w2_d, b2_d, out_d):
    mm = nc.tensor.matmul

    from contextlib import ExitStack
    with ExitStack() as outer:
        consts = outer.enter_context(tc.tile_pool(name="consts", bufs=1))
        mlpre = outer.enter_context(tc.tile_pool(name="mlpre", bufs=1))
        x2 = [mlpre.tile([128, D], BF16, tag=f"x2_{t}", name=f"x2_{t}")
              for t in range(4)]
        h2T = [mlpre.tile([128, TOK], BF16, tag=f"h2T{dc}", name=f"h2T{dc}")
               for dc in range(8)]
        dram = outer.enter_context(tc.tile_pool(name="dram", bufs=1, space="DRAM"))

        # ---- constants
        ident = consts.tile([128, 128], BF16, tag="ident", name="ident")
        make_identity(nc, ident)
        eps_t = consts.tile([128, 1], F32, tag="eps", name="eps")
        nc.vector.memset(eps_t, EPS)
        ones_r = consts.tile([1, 64], BF16, tag="ones_r", name="ones_r")
        nc.vector.memset(ones_r, 1.0)
        bqk_t = consts.tile([128, 4], F32, tag="bqk", name="bqk")
        nc.sync.dma_start(out=bqk_t, in_=bass.AP(
            tensor=bqk_d[:].tensor, offset=0, ap=[[1, 128], [128, 4]]))
        bvbc = consts.tile([128, 256], F32, tag="bvbc", name="bvbc")
        nc.scalar.dma_start(out=bvbc, in_=_bc(bv_d[:]))
        # late-use constants: emitted after the hot startup loads below
        a2bc = consts.tile([128, D], BF16, tag="a2bc", name="a2bc")
        b2bc = consts.tile([128, D], BF16, tag="b2bc", name="b2bc")
        b1_t = consts.tile([128, 32], F32, tag="b1t", name="b1t")

        # ---- DRAM scratch for the chunked collective (one tile per q-block)
        y_part = [dram.tile([512, D], BF16, tag=f"y_part{i}", name=f"y_part{i}")
                  for i in range(4)]
        y_sum = [dram.tile([128, D], BF16, tag=f"y_sum{i}", name=f"y_sum{i}")
                 for i in range(4)]


        w1pool = outer.enter_context(tc.tile_pool(name="w1pool", bufs=1))
        w1_sb = [w1pool.tile([128, 8, 128], BF16, tag=f"w1_{rt}", name=f"w1_{rt}")
                 for rt in range(32)]

        def w1_load(rt):
            nc.scalar.dma_start(
                out=w1_sb[rt], in_=w1_d[rt].rearrange("(kc p) r -> p kc r", p=128))

        mtmp = outer.enter_context(tc.tile_pool(name="mtmp", bufs=1))

        ys_t = [mtmp.tile([128, D], BF16, tag=f"ys{i}", name=f"ys{i}")
                for i in range(2)]
        xr_t = [mtmp.tile([128, D], BF16, tag=f"xr{i}", name=f"xr{i}")
                for i in range(2)]

        def ln2_load(t2):
            nc.gpsimd.dma_start(out=ys_t[t2 % 2], in_=y_sum[t2][:])
            nc.gpsimd.dma_start(out=xr_t[t2 % 2],
                                in_=xr_d[t2 * 128:(t2 + 1) * 128, :])

        def ln2_chunk(t2):
            nc.vector.tensor_tensor(out=x2[t2], in0=xr_t[t2 % 2],
                                    in1=ys_t[t2 % 2], op=ALU.add)
            st2 = mtmp.tile([128, 2, 6], F32, tag="st2", name="st2")
            xg2 = x2[t2].rearrange("p (g d) -> p g d", g=2)
            for g in range(2):
                nc.vector.bn_stats(out=st2[:, g, :], in_=xg2[:, g, :])
            mv2 = mtmp.tile([128, 2], F32, tag="mv2", name="mv2")
            nc.vector.bn_aggr(out=mv2, in_=st2)
            rstd2 = mtmp.tile([128, 1], F32, tag="rstd2", name="rstd2")
            nc.scalar.activation(out=rstd2, in_=mv2[:, 1:2], func=AF.Sqrt,
                                 bias=eps_t, scale=1.0)
            nc.vector.reciprocal(out=rstd2, in_=rstd2)
            xh2 = mtmp.tile([128, D], F32, tag="xh2", name="xh2")
            nc.vector.tensor_scalar(out=xh2, in0=x2[t2],
                                    scalar1=mv2[:, 0:1],
                                    scalar2=rstd2, op0=ALU.subtract,
                                    op1=ALU.mult)
            h2 = mtmp.tile([128, D], BF16, tag="h2", name="h2")
            nc.gpsimd.tensor_tensor(out=h2, in0=xh2, in1=a2bc, op=ALU.mult)
            for dc in range(8):
                nc.sync.dma_start_transpose(
                    out=h2T[dc][:, t2 * 128:(t2 + 1) * 128],
                    in_=h2[:, dc * 128:(dc + 1) * 128])

        with ExitStack() as attctx:
            wpool = attctx.enter_context(tc.tile_pool(name="wpool", bufs=1))
            acts = attctx.enter_context(tc.tile_pool(name="acts", bufs=1))

            wqk_sb = [wpool.tile([128, 512], BF16, tag=f"wqk{k}", name=f"wqk{k}") for k in range(8)]
            wv_sb = [wpool.tile([128, 256], BF16, tag=f"wv{k}", name=f"wv{k}") for k in range(8)]
            wo_sb = [wpool.tile([128, D], BF16, tag=f"wo{k}", name=f"wo{k}") for k in range(2)]

            qkT = [acts.tile([128, S], F8, tag=f"qkT{rt}", name=f"qkT{rt}") for rt in range(4)]
            v_aug = [acts.tile([128, 4, 65], F8, tag=f"vaug{tt}", name=f"vaug{tt}") for tt in range(16)]
            attnT = [acts.tile([128, S], BF16, tag=f"attnT{i}", name=f"attnT{i}") for i in range(2)]

            # ================= P1: load pre-normalized h1^T ==================
            with tc.tile_pool(name="hTp", bufs=1) as hTp:
                hT = [hTp.tile([128, S], BF16, tag=f"hT{dc}", name=f"hT{dc}") for dc in range(8)]
                # interleave weight-chunk and activation-chunk loads so the
                # first projection accumulation chain progresses with the DMAs
                for kc in range(8):
                    nc.sync.dma_start(out=wqk_sb[kc], in_=wqk_d[kc * 128:(kc + 1) * 128, :])
                    nc.scalar.dma_start(out=hT[kc], in_=x_d[kc * 128:(kc + 1) * 128, :])
                for kc in range(8):
                    nc.scalar.dma_start(out=wv_sb[kc], in_=wv_d[kc * 128:(kc + 1) * 128, :])
                for kc in range(2):
                    nc.sync.dma_start(out=wo_sb[kc], in_=wo_d[kc * 128:(kc + 1) * 128, :])
                nc.scalar.dma_start(out=a2bc, in_=_bc(a2_d[:]))
                nc.scalar.dma_start(out=b2bc, in_=_bc(b2_d[:]))
                nc.scalar.dma_start(out=b1_t, in_=bass.AP(
                    tensor=b1_d[:].tensor, offset=0, ap=[[1, 128], [128, 32]]))
                # ================= P2: q,k projection (feature-major) =========
                with tc.tile_pool(name="psQK", bufs=4, space="PSUM") as psQK, \
                     tc.tile_pool(name="psV", bufs=2, space="PSUM") as psV:
                    for tb in range(4):
                        for rt in range(4):
                            pm = psQK.tile([128, 512], F32, tag="pm", name="pm")
                            for kc in range(8):
                                mm(pm, lhsT=wqk_sb[kc][:, rt * 128:(rt + 1) * 128],
                                   rhs=hT[kc][:, tb * 512:(tb + 1) * 512],
                                   start=(kc == 0), stop=(kc == 7))
                            nc.vector.tensor_scalar_add(
                                out=qkT[rt][:, tb * 512:(tb + 1) * 512],
                                in0=pm, scalar1=bqk_t[:, rt:rt + 1])

                    # ============= P3: v projection (token-major) ==============
                    for tt in range(16):
                        pv = psV.tile([128, 256], F32, tag="pmv", name="pmv")
                        for kc in range(8):
                            mm(pv, lhsT=hT[kc][:, tt * 128:(tt + 1) * 128],
                               rhs=wv_sb[kc], start=(kc == 0), stop=(kc == 7))
                        nc.vector.memset(v_aug[tt], 1.0)
                        nc.vector.tensor_tensor(
                            out=v_aug[tt][:, :, 0:64],
                            in0=pv.rearrange("p (h d) -> p h d", h=4),
                            in1=bvbc.rearrange("p (h d) -> p h d", h=4),
                            op=ALU.add)

            # ================= P4/P5: attention + out-projection ==============
            with tc.tile_pool(name="attp", bufs=1) as attp, \
                 tc.tile_pool(name="att2", bufs=2) as att2, \
                 tc.tile_pool(name="psS", bufs=1, space="PSUM") as psS, \
                 tc.tile_pool(name="psN01", bufs=2, space="PSUM") as psN01, \
                 tc.tile_pool(name="psN23", bufs=2, space="PSUM") as psN23, \
                 tc.tile_pool(name="psO", bufs=2, space="PSUM") as psO:
                def dr2(ap):
                    # [2k, n] view as DoubleRow [k, 2, n] (paired contraction)
                    return ap.rearrange("(i p) n -> p i n", i=2)

                def scores_kt(qb, pair, kt, exp_t):
                    qsl = slice(qb * 512, (qb + 1) * 512)
                    for sub in range(2):
                        h = 2 * pair + sub
                        psl = slice(sub * 64, (sub + 1) * 64)
                        ps = psS.tile([128, 512], F32, tag=f"scr{sub}",
                                      name=f"scr{sub}")
                        mm(ps, lhsT=dr2(qkT[2 + pair][psl, kt * 128:(kt + 1) * 128]),
                           rhs=dr2(qkT[pair][psl, qsl]),
                           start=True, stop=True, tile_position=(sub * 64, 0),
                           perf_mode=DROW)
                        if (2 * kt + sub) % 16 < 6:
                            # Schraudolph fp8 exp on DVE (ACT offload)
                            ei = attp.tile([128, 512], mybir.dt.int8,
                                           tag=f"e{sub}_{kt}", name=f"e{sub}_{kt}")
                            nc.vector.tensor_scalar(
                                out=ei, in0=ps, scalar1=1.442695,
                                scalar2=55.5, op0=ALU.mult, op1=ALU.add)
                            exp_t[(h, kt)] = ei[:].bitcast(F8)
                        else:
                            ex = attp.tile([128, 512], F8, tag=f"e{sub}_{kt}",
                                           name=f"e{sub}_{kt}")
                            nc.scalar.activation(out=ex, in_=ps, func=AF.Exp,
                                                 scale=0.125)
                            exp_t[(h, kt)] = ex

                def attnv_kt(pn, h, kt, exp_t):
                    mm(pn[0:65, :], lhsT=dr2(v_aug[kt][:, h, :]),
                       rhs=dr2(exp_t[(h, kt)]),
                       start=(kt == 0), stop=(kt == 15), perf_mode=DROW)

                def denom(qb, h, pn):
                    # broadcast 1/denom into the numerator bank's free rows
                    # 64-127 (the ones-row at 64 is consumed first), so the
                    # divide reads two halves of one PSUM bank.
                    qsl = slice(qb * 512, (qb + 1) * 512)
                    rc = att2.tile([1, 512], BF16, tag="rc", name="rc")
                    with nc.allow_low_precision(reason="bf16 softmax denom"):
                        nc.vector.reciprocal(out=rc, in_=pn[64:65, :])
                    mm(pn[64:128, :], lhsT=ones_r, rhs=rc, start=True, stop=True,
                       tile_position=(0, 64))
                    rcb = att2.tile([64, 512], BF16, tag="rcb", name="rcb")
                    nc.scalar.copy(out=rcb, in_=pn[64:128, :])
                    if h % 2 == 0:
                        nc.vector.tensor_tensor(
                            out=attnT[h // 2][0:64, qsl],
                            in0=pn[0:64, :], in1=rcb, op=ALU.mult)
                    else:
                        ad = att2.tile([64, 512], BF16, tag="adiv", name="adiv")
                        nc.vector.tensor_tensor(
                            out=ad, in0=pn[0:64, :], in1=rcb, op=ALU.mult)
                        nc.sync.dma_start(
                            out=attnT[h // 2][64:128, qsl], in_=ad)

                def outproj_rs(qb):
                    for tt in range(4):
                        tok = qb * 512 + tt * 128
                        yb = att2.tile([128, D], BF16, tag="ysb", name="ysb")
                        for n in range(2):
                            po = psO.tile([128, 512], F32, tag="po", name="po")
                            for kc in range(2):
                                mm(po, lhsT=attnT[kc][:, tok:tok + 128],
                                   rhs=wo_sb[kc][:, n * 512:(n + 1) * 512],
                                   start=(kc == 0), stop=(kc == 1))
                            if n == 0:
                                nc.vector.tensor_copy(out=yb[:, 0:512], in_=po)
                            else:
                                nc.scalar.copy(out=yb[:, 512:1024], in_=po)
                        nc.sync.dma_start(
                            out=y_part[qb][tt * 128:(tt + 1) * 128, :], in_=yb)
                    nc.gpsimd.collective_compute(
                        "ReduceScatter", ALU.add, replica_groups=GROUPS,
                        ins=[y_part[qb].opt()], outs=[y_sum[qb].opt()])

                # Software-pipelined attention: pair-p scores interleave with
                # the previous pair's attn@V accumulation so the PE never
                # stalls on softmax-exp backpressure.
                exp_store = {}
                pns = {}
                for qb in range(4):
                    exp_t = {}
                    exp_store[qb] = exp_t
                    if qb > 0:
                        pns[(qb - 1, 2)] = psN23.tile([128, 512], F32, tag="num",
                                                      name="num")
                        pns[(qb - 1, 3)] = psN23.tile([128, 512], F32, tag="num",
                                                      name="num")
                    for kt in range(16):
                        if qb > 0:
                            attnv_kt(pns[(qb - 1, 2)], 2, kt, exp_store[qb - 1])
                            attnv_kt(pns[(qb - 1, 3)], 3, kt, exp_store[qb - 1])
                        scores_kt(qb, 0, kt, exp_t)
                    if qb > 0:
                        denom(qb - 1, 2, pns[(qb - 1, 2)])
                        denom(qb - 1, 3, pns[(qb - 1, 3)])
                        outproj_rs(qb - 1)
                        if qb > 1:
                            ln2_chunk(qb - 2)
                    pns[(qb, 0)] = psN01.tile([128, 512], F32, tag="num", name="num")
                    pns[(qb, 1)] = psN01.tile([128, 512], F32, tag="num", name="num")
                    for kt in range(16):
                        attnv_kt(pns[(qb, 0)], 0, kt, exp_t)
                        attnv_kt(pns[(qb, 1)], 1, kt, exp_t)
                        scores_kt(qb, 1, kt, exp_t)
                    denom(qb, 0, pns[(qb, 0)])
                    denom(qb, 1, pns[(qb, 1)])
                    if qb >= 1:
                        ln2_load(qb - 1)
                    if qb >= 2:
                        # prefetch mlp_w1 while attention DMA queues are quiet
                        for rt in range((qb - 2) * 16, (qb - 1) * 16):
                            w1_load(rt)
                # drain: last qb's heads 2,3 + out-proj + RS + remaining ln2
                pns[(3, 2)] = psN23.tile([128, 512], F32, tag="num", name="num")
                pns[(3, 3)] = psN23.tile([128, 512], F32, tag="num", name="num")
                for kt in range(16):
                    attnv_kt(pns[(3, 2)], 2, kt, exp_store[3])
                    attnv_kt(pns[(3, 3)], 3, kt, exp_store[3])
                denom(3, 2, pns[(3, 2)])
                denom(3, 3, pns[(3, 3)])
                outproj_rs(3)
                ln2_chunk(2)
                ln2_load(3)
        with tc.tile_pool(name="mlpp", bufs=1) as mlpp, \
             tc.tile_pool(name="w2pool", bufs=1) as w2pool, \
             tc.tile_pool(name="mlptmp", bufs=2) as mlptmp:
            w2_sb2 = [w2pool.tile([128, D], BF16, tag=f"w2_{kc}",
                                  name=f"w2_{kc}") for kc in range(32)]
            g1T = [mlpp.tile([128, TOK], BF16, tag=f"g1T{rt}", name=f"g1T{rt}") for rt in range(32)]

            for kc in range(32):
                nc.sync.dma_start(out=w2_sb2[kc],
                                  in_=w2_d[kc * 128:(kc + 1) * 128, :])

            def mlp_w1_pass(psM1, half):
                # tokens [half*256, half*256+256): half 0 needs only ln2(0,1),
                # so it runs while the last ReduceScatter is still in flight
                csl = slice(half * 256, half * 256 + 256)
                for rt in range(32):
                    pm1 = psM1.tile([128, 256], F32, tag="pm1", name="pm1")
                    for kc in range(8):
                        mm(pm1, lhsT=w1_sb[rt][:, kc, :], rhs=h2T[kc][:, csl],
                           start=(kc == 0), stop=(kc == 7))
                    nc.scalar.activation(out=g1T[rt][:, csl], in_=pm1,
                                         func=AF.Gelu_apprx_tanh,
                                         bias=b1_t[:, rt:rt + 1], scale=1.0)

            def mlp_w2_t2(psM2, t2):
                ob = mlptmp.tile([128, D], F32, tag="ob", name="ob")
                for n in range(2):
                    nsl = slice(n * 512, (n + 1) * 512)
                    pm2 = psM2.tile([128, 512], F32, tag="pm2", name="pm2")
                    for kc in range(32):
                        mm(pm2, lhsT=g1T[kc][:, t2 * 128:(t2 + 1) * 128],
                           rhs=w2_sb2[kc][:, nsl],
                           start=(kc == 0), stop=(kc == 31))
                    tb = mlptmp.tile([128, 512], F32, tag="tb9", name="tb9")
                    nc.vector.tensor_tensor(out=tb, in0=pm2, in1=b2bc[:, nsl],
                                            op=ALU.add)
                    nc.vector.tensor_tensor(out=ob[:, nsl], in0=tb,
                                            in1=x2[t2][:, nsl], op=ALU.add)
                nc.sync.dma_start(out=out_d[t2 * 128:(t2 + 1) * 128, :], in_=ob)

            with tc.tile_pool(name="psM1", bufs=4, space="PSUM") as psM1, \
                 tc.tile_pool(name="psM2", bufs=2, space="PSUM") as psM2:
                mlp_w1_pass(psM1, 0)
                ln2_chunk(3)
                mlp_w2_t2(psM2, 0)
                mlp_w2_t2(psM2, 1)
                mlp_w1_pass(psM1, 1)
                mlp_w2_t2(psM2, 2)
                mlp_w2_t2(psM2, 3)


# ----------------------------------------------------------------- kernel()

def _get_nc():
    if "nc" not in _CACHE:
        _CACHE["nc"] = build_program()
    return _CACHE["nc"]


def kernel(**inputs) -> np.ndarray:
    in_maps = host_prep(inputs)
    nc = _get_nc()
    res = run_bass_kernel_spmd(nc, in_maps, list(range(8)))
    out = np.zeros((B, S, D), np.float32)
    for cid in range(8):
        b, r = cid // 4, cid % 4
        o = res.results[cid]["out"]
        for t2 in range(4):
            out[b, 512 * t2 + 128 * r:512 * t2 + 128 * r + 128] = \
                o[128 * t2:128 * t2 + 128]
    return out

